# revision 5
# baseline (speedup 1.0000x reference)
"""Trainium2 Bass kernel for nn_LlamaMoDDecoderLayer (MoD decoder layer).

v2 strategy (8 NeuronCores, tensor-parallel, feature-major layouts):
  - Host precompute (free): router argmax masks in exact fp32; r1 (RMSNorm1
    row scales) folded into the RoPE cos/sin tables (q/k) and a per-token
    column vector (v); hsT shipped once in bf16; ln weights folded into Wq/
    Wk/Wv/w_gate/w_up.
  - QKV runs directly on raw bf16 hsT (the RMS column scale commutes out of
    the matmul); heads sharded 2/core; transposed-scores causal softmax with
    denominator accumulated on DVE (single ones-matmul per (h,chunk)).
  - Pipeline over 2 token-chunks of 1024: attn -> AG(ctx) -> Wo -> AG(hs2)
    -> norm2+MLP -> RS -> out, with per-chunk collectives overlapping PE.
  - MLP: w_gate/w_up column-sharded, w_down row-sharded, partial outputs
    ReduceScattered per chunk.
  - Matmuls bf16 (host-cast weights), fp32 PSUM accumulation; fp32 residual.
"""

import numpy as np
import ml_dtypes

import concourse.bass as bass
import concourse.bacc as bacc
import concourse.mybir as mybir
import concourse.tile as tile
from concourse.alu_op_type import AluOpType
from concourse.bass_utils import run_bass_kernel_spmd

F32 = mybir.dt.float32
BF16 = mybir.dt.bfloat16
AF = mybir.ActivationFunctionType

S, D, H, Dh, F = 2048, 2048, 16, 128, 8192
NC = 8
HPC = H // NC            # heads per core (2)
DCC = D // NC            # output cols per core (256)
FPC = F // NC            # mlp hidden per core (1024)
NDT = D // 128           # 16 d-tiles
NFT = FPC // 128         # 8 local f-tiles
NSC = S // 512           # 4 s-chunks of 512
NPC = 2                  # pipeline chunks
SC = S // NPC            # 1024 tokens per pipeline chunk
NQC = SC // 512          # 512-token q-subchunks per pipeline chunk
EPS = 1e-5
THETA = 10000.0

_CACHE = {}


def _build_program():
    nc = bacc.Bacc("TRN2", target_bir_lowering=False, debug=False,
                   num_devices=NC)
    rg = [list(range(NC))]

    d_bht = nc.dram_tensor("bht", [D, S], BF16, kind="ExternalInput")
    d_hres = nc.dram_tensor("hres", [DCC, S], F32, kind="ExternalInput")
    d_wq = nc.dram_tensor("wq", [D, DCC], BF16, kind="ExternalInput")
    d_wk = nc.dram_tensor("wk", [D, DCC], BF16, kind="ExternalInput")
    d_wv = nc.dram_tensor("wv", [D, DCC], BF16, kind="ExternalInput")
    d_wo = nc.dram_tensor("wo", [D, DCC], BF16, kind="ExternalInput")
    d_wg = nc.dram_tensor("wg", [D, FPC], BF16, kind="ExternalInput")
    d_wu = nc.dram_tensor("wu", [D, FPC], BF16, kind="ExternalInput")
    d_wd = nc.dram_tensor("wd", [FPC, D], BF16, kind="ExternalInput")
    d_qcos = nc.dram_tensor("qcos", [Dh, S], BF16, kind="ExternalInput")
    d_qsin = nc.dram_tensor("qsin", [Dh, S], BF16, kind="ExternalInput")
    d_kcos = nc.dram_tensor("kcos", [Dh, S], BF16, kind="ExternalInput")
    d_ksin = nc.dram_tensor("ksin", [Dh, S], BF16, kind="ExternalInput")
    d_tri = nc.dram_tensor("tri", [128, 4 * 512], BF16, kind="ExternalInput")
    d_ma = nc.dram_tensor("ma", [128, S], BF16, kind="ExternalInput")
    d_mm = nc.dram_tensor("mm", [128, S], BF16, kind="ExternalInput")
    d_r1c = nc.dram_tensor("r1c", [128, NDT], F32, kind="ExternalInput")
    d_out = nc.dram_tensor("out", [DCC, S], F32, kind="ExternalOutput")

    cc1i, cc1o, cc2i, cc2o, cc3i, cc3o = [], [], [], [], [], []
    for pc in range(NPC):
        cc1i.append(nc.dram_tensor(f"cc1i{pc}", [DCC, SC], BF16))
        cc1o.append(nc.dram_tensor(f"cc1o{pc}", [D, SC], BF16,
                                   addr_space="Shared"))
        cc2i.append(nc.dram_tensor(f"cc2i{pc}", [DCC, SC], BF16))
        cc2o.append(nc.dram_tensor(f"cc2o{pc}", [D, SC], BF16,
                                   addr_space="Shared"))
        cc3i.append(nc.dram_tensor(f"cc3i{pc}", [D, SC], BF16))
        cc3o.append(nc.dram_tensor(f"cc3o{pc}", [DCC, SC], BF16))

    bht_t = d_bht.ap().rearrange("(a p) s -> p a s", p=128)
    hres_t = d_hres.ap().rearrange("(a p) s -> p a s", p=128)
    wq_t = d_wq.ap().rearrange("(a p) m -> p a m", p=128)
    wk_t = d_wk.ap().rearrange("(a p) m -> p a m", p=128)
    wv_t = d_wv.ap().rearrange("(a p) m -> p a m", p=128)
    wo_t = d_wo.ap().rearrange("(a p) m -> p a m", p=128)
    wg_t = d_wg.ap().rearrange("(a p) m -> p a m", p=128)
    wu_t = d_wu.ap().rearrange("(a p) m -> p a m", p=128)
    wd_t = d_wd.ap().rearrange("(a p) m -> p a m", p=128)
    cc1i_t = [t.ap().rearrange("(a p) s -> p a s", p=128) for t in cc1i]
    cc1o_t = [t.ap().rearrange("(a p) s -> p a s", p=128) for t in cc1o]
    cc2i_t = [t.ap().rearrange("(a p) s -> p a s", p=128) for t in cc2i]
    cc2o_t = [t.ap().rearrange("(a p) s -> p a s", p=128) for t in cc2o]
    cc3i_t = [t.ap().rearrange("(a p) s -> p a s", p=128) for t in cc3i]
    cc3o_t = [t.ap().rearrange("(a p) s -> p a s", p=128) for t in cc3o]
    out_t = d_out.ap().rearrange("(a p) s -> p a s", p=128)

    with tile.TileContext(nc) as tc:
        with (
            tc.tile_pool(name="const", bufs=1) as cst,
            tc.tile_pool(name="persist", bufs=1) as pst,
            tc.tile_pool(name="psum", bufs=2, space="PSUM") as psp,
        ):
            ones_b = cst.tile([128, 1], BF16)
            nc.gpsimd.memset(ones_b[:], 1.0)
            ones_r = cst.tile([1, 128], F32)
            nc.gpsimd.memset(ones_r[:], 1.0)
            eps1 = cst.tile([1, 1], F32)
            nc.gpsimd.memset(eps1[:], EPS)
            r1c = cst.tile([128, NDT], F32, name="r1c")
            nc.sync.dma_start(r1c[:], d_r1c.ap())
            ma_b = pst.tile([128, S], BF16, name="ma_b")
            mm_b = pst.tile([128, S], BF16, name="mm_b")
            nc.sync.dma_start(ma_b[:], d_ma.ap())
            nc.sync.dma_start(mm_b[:], d_mm.ap())
            # hs2f doubles as the residual: loaded with hres, Wo adds into it
            hs2f = pst.tile([128, HPC, S], F32, name="hs2f")
            nc.sync.dma_start(hs2f[:], hres_t)
            wo = pst.tile([128, NDT, DCC], BF16, name="wo")
            nc.scalar.dma_start(wo[:], wo_t)

            with tc.tile_pool(name="attn", bufs=1) as atp:
                qr = atp.tile([128, HPC, S], BF16, name="qr")
                kr = atp.tile([128, HPC, S], BF16, name="kr")
                v_sb = atp.tile([128, NDT, DCC], BF16, name="v_sb")
                tri = atp.tile([128, 4, 512], BF16, name="tri")
                nc.sync.dma_start(
                    tri[:], d_tri.ap().rearrange("p (a m) -> p a m", m=512))
                q_sb = atp.tile([128, HPC, S], BF16, name="q_sb")
                k_sb = atp.tile([128, HPC, S], BF16, name="k_sb")
                qcos = atp.tile([128, S], BF16, name="qcos")
                qsin = atp.tile([128, S], BF16, name="qsin")
                kcos = atp.tile([128, S], BF16, name="kcos")
                ksin = atp.tile([128, S], BF16, name="ksin")
                nc.sync.dma_start(qcos[:], d_qcos.ap())
                nc.sync.dma_start(qsin[:], d_qsin.ap())
                nc.sync.dma_start(kcos[:], d_kcos.ap())
                nc.sync.dma_start(ksin[:], d_ksin.ap())

                # ---- QKV on raw bht (two S-halves); r1 folded into
                #      rope tables (q,k) and r1c (v) ----
                with tc.tile_pool(name="qk", bufs=1) as qkp:
                    wq = qkp.tile([128, NDT, DCC], BF16, name="wq")
                    wk = qkp.tile([128, NDT, DCC], BF16, name="wk")
                    wv = qkp.tile([128, NDT, DCC], BF16, name="wv")
                    nc.scalar.dma_start(wq[:], wq_t)
                    nc.scalar.dma_start(wk[:], wk_t)
                    nc.scalar.dma_start(wv[:], wv_t)
                    for sh in range(2):
                        bh = qkp.tile([128, NDT, S // 2], BF16, tag="bht",
                                      bufs=2)
                        for a in range(NDT):
                            nc.sync.dma_start(
                                bh[:, a, :],
                                bht_t[:, a, bass.ts(sh, S // 2)])
                        for w_sb, t_sb in ((wq, q_sb), (wk, k_sb)):
                            for mc in range(HPC):
                                for si in range(2):
                                    sc = sh * 2 + si
                                    ps = psp.tile([128, 512], F32, tag="mmps")
                                    for a in range(NDT):
                                        nc.tensor.matmul(
                                            ps[:],
                                            w_sb[:, a, bass.ts(mc, 128)],
                                            bh[:, a, bass.ts(si, 512)],
                                            start=(a == 0),
                                            stop=(a == NDT - 1))
                                    nc.scalar.copy(
                                        t_sb[:, mc, bass.ts(sc, 512)], ps[:])
                        for mi in range(NDT // 2):
                            mc = sh * (NDT // 2) + mi
                            ps = psp.tile([128, DCC], F32, tag="vps", bufs=1)
                            for a in range(NDT):
                                nc.tensor.matmul(
                                    ps[:], bh[:, a, bass.ts(mi, 128)],
                                    wv[:, a, :],
                                    start=(a == 0), stop=(a == NDT - 1))
                            nc.vector.tensor_scalar(
                                v_sb[:, mc, :], ps[:], r1c[:, mc:mc + 1],
                                None, op0=AluOpType.mult)

                with tc.tile_pool(name="rope", bufs=1) as rpp:
                    qs_sb = rpp.tile([128, HPC, S], BF16, name="qs_sb")
                    ks_sb = rpp.tile([128, HPC, S], BF16, name="ks_sb")
                    for src, dst in ((q_sb, qs_sb), (k_sb, ks_sb)):
                        for mc in range(HPC):
                            nc.sync.dma_start(dst[0:64, mc, :],
                                              src[64:128, mc, :])
                            nc.sync.dma_start(dst[64:128, mc, :],
                                              src[0:64, mc, :])
                    for mc in range(HPC):
                        tq = rpp.tile([128, S], BF16, tag="ropetmp", bufs=2)
                        nc.vector.tensor_tensor(tq[:], qs_sb[:, mc, :],
                                                qsin[:], op=AluOpType.mult)
                        nc.vector.tensor_tensor(qr[:, mc, :], q_sb[:, mc, :],
                                                qcos[:], op=AluOpType.mult)
                        nc.vector.tensor_tensor(qr[:, mc, :], qr[:, mc, :],
                                                tq[:], op=AluOpType.add)
                        tk = rpp.tile([128, S], BF16, tag="ropetmp", bufs=2)
                        nc.vector.tensor_tensor(tk[:], ks_sb[:, mc, :],
                                                ksin[:], op=AluOpType.mult)
                        nc.vector.tensor_tensor(kr[:, mc, :], k_sb[:, mc, :],
                                                kcos[:], op=AluOpType.mult)
                        nc.vector.tensor_tensor(kr[:, mc, :], kr[:, mc, :],
                                                tk[:], op=AluOpType.add)

                # ---- attention: causal softmax, denominator on DVE ----
                for pc in range(NPC):
                    for qs_i in range(NQC):
                        qc = pc * NQC + qs_i
                        for h in range(HPC):
                            nkt = 4 * (qc + 1)
                            cps = psp.tile([128, 512], F32, tag="ctxps",
                                           bufs=1)
                            dsum = atp.tile([128, 512], BF16, tag="dsum",
                                            bufs=2)
                            for kt in range(nkt):
                                sps = psp.tile([128, 512], F32, tag="stps")
                                nc.tensor.matmul(sps[:],
                                                 kr[:, h, bass.ts(kt, 128)],
                                                 qr[:, h, bass.ts(qc, 512)])
                                est = atp.tile([128, 512], BF16, tag="est",
                                               bufs=3)
                                nc.scalar.activation(est[:], sps[:], AF.Exp)
                                if kt // 4 == qc:
                                    nc.vector.tensor_tensor(
                                        est[:], est[:], tri[:, kt % 4, :],
                                        op=AluOpType.mult)
                                nc.tensor.matmul(cps[:],
                                                 v_sb[:, kt, bass.ts(h, 128)],
                                                 est[:], start=(kt == 0),
                                                 stop=(kt == nkt - 1))
                                if kt == 0:
                                    nc.vector.tensor_copy(dsum[:], est[:])
                                else:
                                    nc.vector.tensor_tensor(
                                        dsum[:], dsum[:], est[:],
                                        op=AluOpType.add)
                            dps = psp.tile([1, 512], F32, tag="rowps",
                                           bufs=1)
                            nc.tensor.matmul(dps[:], ones_b[:], dsum[:])
                            rrow = atp.tile([1, 512], F32, tag="rrow", bufs=2)
                            nc.vector.reciprocal(rrow[:], dps[:])
                            bps = psp.tile([128, 512], F32, tag="bcps",
                                           bufs=1)
                            nc.tensor.matmul(bps[:], ones_r[:], rrow[:])
                            rb = atp.tile([128, 512], F32, tag="rb", bufs=2)
                            nc.scalar.copy(rb[:], bps[:])
                            ctxc = atp.tile([128, 512], BF16, tag="ctxc",
                                            bufs=2)
                            nc.vector.tensor_tensor(ctxc[:], cps[:], rb[:],
                                                    op=AluOpType.mult)
                            nc.sync.dma_start(
                                cc1i_t[pc][:, h, bass.ts(qs_i, 512)],
                                ctxc[:])
                    nc.gpsimd.collective_compute(
                        "AllGather", AluOpType.bypass, replica_groups=rg,
                        ins=[cc1i[pc].ap()], outs=[cc1o[pc].ap()])

            # attention SBUF freed: stream MLP weights into that space
            with tc.tile_pool(name="mlw", bufs=1) as mlw:
                wg = mlw.tile([128, NDT, FPC], BF16, name="wg")
                wu = mlw.tile([128, NDT, FPC], BF16, name="wu")
                nc.scalar.dma_start(wg[:], wg_t)
                nc.scalar.dma_start(wu[:], wu_t)

                # ---- Wo + hs2 per chunk ----
                with tc.tile_pool(name="wop", bufs=1) as wop:
                    for pc in range(NPC):
                        ctxg = wop.tile([128, NDT, SC], BF16, tag="ctxg",
                                        bufs=2)
                        nc.sync.dma_start(ctxg[:], cc1o_t[pc])
                        for mc in range(HPC):
                            for sc_i in range(NQC):
                                col = pc * SC + sc_i * 512
                                ps = psp.tile([128, 512], F32, tag="mmps")
                                for a in range(NDT):
                                    nc.tensor.matmul(
                                        ps[:], wo[:, a, bass.ts(mc, 128)],
                                        ctxg[:, a, bass.ts(sc_i, 512)],
                                        start=(a == 0), stop=(a == NDT - 1))
                                t = wop.tile([128, 512], F32, tag="wot",
                                             bufs=2)
                                nc.vector.tensor_tensor(
                                    t[:], ps[:], ma_b[:, col:col + 512],
                                    op=AluOpType.mult)
                                nc.vector.tensor_tensor(
                                    hs2f[:, mc, col:col + 512], t[:],
                                    hs2f[:, mc, col:col + 512],
                                    op=AluOpType.add)
                                hs2c = wop.tile([128, 512], BF16, tag="hs2c",
                                                bufs=2)
                                nc.scalar.copy(hs2c[:],
                                               hs2f[:, mc, col:col + 512])
                                nc.sync.dma_start(
                                    cc2i_t[pc][:, mc, bass.ts(sc_i, 512)],
                                    hs2c[:])
                        nc.gpsimd.collective_compute(
                            "AllGather", AluOpType.bypass, replica_groups=rg,
                            ins=[cc2i[pc].ap()], outs=[cc2o[pc].ap()])

                # ---- norm2 + MLP + RS + out per chunk ----
                with tc.tile_pool(name="mlp", bufs=1) as mlp:
                    for pc in range(NPC):
                        hs2g = mlp.tile([128, NDT, SC], BF16, tag="hs2g",
                                        bufs=1)
                        nc.sync.dma_start(hs2g[:], cc2o_t[pc])
                        r2b = mlp.tile([128, SC], F32, tag="r2b", bufs=2)
                        for sc_i in range(NQC):
                            rps = psp.tile([1, 512], F32, tag="rowps",
                                           bufs=1)
                            for a in range(NDT):
                                sq = mlp.tile([128, 512], BF16, tag="sq",
                                              bufs=3)
                                nc.scalar.activation(
                                    sq[:], hs2g[:, a, bass.ts(sc_i, 512)],
                                    AF.Square)
                                nc.tensor.matmul(rps[:], ones_b[:], sq[:],
                                                 start=(a == 0),
                                                 stop=(a == NDT - 1))
                            r2row = mlp.tile([1, 512], F32, tag="r2row",
                                             bufs=2)
                            nc.scalar.activation(r2row[:], rps[:], AF.Sqrt,
                                                 bias=eps1[:], scale=1.0 / D)
                            nc.vector.reciprocal(r2row[:], r2row[:])
                            bps = psp.tile([128, 512], F32, tag="bcps",
                                           bufs=1)
                            nc.tensor.matmul(bps[:], ones_r[:], r2row[:])
                            nc.scalar.copy(r2b[:, bass.ts(sc_i, 512)], bps[:])
                        for a in range(NDT):
                            nc.vector.tensor_tensor(
                                hs2g[:, a, :], hs2g[:, a, :], r2b[:],
                                op=AluOpType.mult)
                        hT = mlp.tile([128, NFT, SC], BF16, tag="hT", bufs=1)
                        for fc in range(NFT):
                            sg = mlp.tile([128, SC], BF16, tag="sg", bufs=2)
                            for sc_i in range(NQC):
                                ps = psp.tile([128, 512], F32, tag="mmps")
                                for a in range(NDT):
                                    nc.tensor.matmul(
                                        ps[:], wg[:, a, bass.ts(fc, 128)],
                                        hs2g[:, a, bass.ts(sc_i, 512)],
                                        start=(a == 0), stop=(a == NDT - 1))
                                nc.scalar.activation(
                                    sg[:, bass.ts(sc_i, 512)], ps[:],
                                    AF.Silu)
                            for sc_i in range(NQC):
                                ps = psp.tile([128, 512], F32, tag="mmps")
                                for a in range(NDT):
                                    nc.tensor.matmul(
                                        ps[:], wu[:, a, bass.ts(fc, 128)],
                                        hs2g[:, a, bass.ts(sc_i, 512)],
                                        start=(a == 0), stop=(a == NDT - 1))
                                nc.vector.tensor_tensor(
                                    hT[:, fc, bass.ts(sc_i, 512)], ps[:],
                                    sg[:, bass.ts(sc_i, 512)],
                                    op=AluOpType.mult)
                        for mc in range(NDT):
                            wdc = mlp.tile([128, NFT, 128], BF16, tag="wdc",
                                           bufs=3)
                            nc.scalar.dma_start(wdc[:],
                                                wd_t[:, :, bass.ts(mc, 128)])
                            for sc_i in range(NQC):
                                ps = psp.tile([128, 512], F32, tag="mmps")
                                for a in range(NFT):
                                    nc.tensor.matmul(
                                        ps[:], wdc[:, a, :],
                                        hT[:, a, bass.ts(sc_i, 512)],
                                        start=(a == 0), stop=(a == NFT - 1))
                                stg = mlp.tile([128, 512], BF16, tag="stg",
                                               bufs=3)
                                nc.scalar.copy(stg[:], ps[:])
                                nc.sync.dma_start(
                                    cc3i_t[pc][:, mc, bass.ts(sc_i, 512)],
                                    stg[:])
                        nc.gpsimd.collective_compute(
                            "ReduceScatter", AluOpType.add, replica_groups=rg,
                            ins=[cc3i[pc].ap()], outs=[cc3o[pc].ap()])

                        rs = mlp.tile([128, HPC, SC], BF16, tag="rs", bufs=2)
                        nc.sync.dma_start(rs[:], cc3o_t[pc])
                        for mc in range(HPC):
                            col = pc * SC
                            t2 = mlp.tile([128, SC], F32, tag="fint", bufs=2)
                            nc.vector.tensor_tensor(t2[:], rs[:, mc, :],
                                                    mm_b[:, col:col + SC],
                                                    op=AluOpType.mult)
                            outt = mlp.tile([128, SC], F32, tag="outt",
                                            bufs=2)
                            nc.vector.tensor_tensor(
                                outt[:], t2[:], hs2f[:, mc, col:col + SC],
                                op=AluOpType.add)
                            nc.sync.dma_start(
                                out_t[:, mc, col:col + SC], outt[:])

    nc.compile()
    return nc


def _rope_tables():
    pos = np.arange(S, dtype=np.float32)
    inv = 1.0 / (THETA ** (np.arange(0, Dh, 2, dtype=np.float32) / Dh))
    ang = pos[:, None] * inv[None, :]
    emb = np.concatenate([ang, ang], axis=-1)          # [S, Dh]
    cosT = np.cos(emb).T.astype(np.float32).copy()     # [Dh, S]
    ssinT = np.sin(emb).T.astype(np.float32).copy()
    ssinT[:64] = -ssinT[:64]
    return cosT, ssinT


def _tri_masks():
    # [128, 4, 512] for the diagonal 512-q-chunk, k-tile offset i in chunk:
    # col j: 0 if j < 128i; causal tri inside diag block; 1 past it.
    m = np.zeros((128, 4, 512), np.float32)
    for i in range(4):
        j = np.arange(512)[None, :]
        p = np.arange(128)[:, None]
        m[:, i, :] = ((j - 128 * i) >= p).astype(np.float32)
        m[:, i, : 128 * i] = 0.0
        m[:, i, 128 * (i + 1):] = 1.0
    return m.reshape(128, 4 * 512)


def kernel(**inputs):
    bf = ml_dtypes.bfloat16
    hs = np.ascontiguousarray(np.asarray(inputs["hidden_states"],
                                         np.float32)[0])
    ln1 = np.asarray(inputs["ln1_w"], np.float32)
    ln2 = np.asarray(inputs["ln2_w"], np.float32)
    Wq = np.asarray(inputs["Wq"], np.float32) * ln1[:, None]
    Wk = np.asarray(inputs["Wk"], np.float32) * ln1[:, None]
    Wv = np.asarray(inputs["Wv"], np.float32) * ln1[:, None]
    Wo = np.asarray(inputs["Wo"], np.float32)
    wg = np.asarray(inputs["w_gate"], np.float32) * ln2[:, None]
    wu = np.asarray(inputs["w_up"], np.float32) * ln2[:, None]
    wd = np.asarray(inputs["w_down"], np.float32)
    raw = np.asarray(inputs["router_attn_w"], np.float32)
    rab = np.asarray(inputs["router_attn_b"], np.float32)
    rmw = np.asarray(inputs["router_mlp_w"], np.float32)
    rmb = np.asarray(inputs["router_mlp_b"], np.float32)

    hsT = np.ascontiguousarray(hs.T)                   # [D, S]

    # routers on host, exact fp32 semantics (keep = argmax == 0)
    al = hs @ raw + rab
    ml_ = hs @ rmw + rmb
    keep_a = (al[:, 1] <= al[:, 0]).astype(np.float32)      # [S]
    keep_m = (ml_[:, 1] <= ml_[:, 0]).astype(np.float32)
    ma = np.ascontiguousarray(
        np.broadcast_to(keep_a[None, :], (128, S)).astype(bf))
    mm = np.ascontiguousarray(
        np.broadcast_to(keep_m[None, :], (128, S)).astype(bf))

    # RMSNorm1 row scales, folded into rope tables (q,k) and r1c (v)
    r1 = (1.0 / np.sqrt((hsT * hsT).mean(0) + EPS)).astype(np.float32)  # [S]
    cosT, ssinT = _rope_tables()
    sc = np.float32(1.0 / np.sqrt(Dh))
    qcos = np.ascontiguousarray((cosT * r1[None, :]).astype(bf))
    qsin = np.ascontiguousarray((ssinT * r1[None, :]).astype(bf))
    kcos = np.ascontiguousarray((cosT * (r1 * sc)[None, :]).astype(bf))
    ksin = np.ascontiguousarray((ssinT * (r1 * sc)[None, :]).astype(bf))
    r1c = np.ascontiguousarray(r1.reshape(NDT, 128).T)  # [128, 16]

    tri = np.ascontiguousarray(_tri_masks().astype(bf))
    bht = np.ascontiguousarray(hsT.astype(bf))

    if "nc" not in _CACHE:
        _CACHE["nc"] = _build_program()
    nc = _CACHE["nc"]

    in_maps = []
    for c in range(NC):
        dsl = slice(c * DCC, (c + 1) * DCC)
        fsl = slice(c * FPC, (c + 1) * FPC)
        in_maps.append({
            "bht": bht,
            "hres": np.ascontiguousarray(hsT[dsl]),
            "wq": np.ascontiguousarray(Wq[:, dsl].astype(bf)),
            "wk": np.ascontiguousarray(Wk[:, dsl].astype(bf)),
            "wv": np.ascontiguousarray(Wv[:, dsl].astype(bf)),
            "wo": np.ascontiguousarray(Wo[:, dsl].astype(bf)),
            "wg": np.ascontiguousarray(wg[:, fsl].astype(bf)),
            "wu": np.ascontiguousarray(wu[:, fsl].astype(bf)),
            "wd": np.ascontiguousarray(wd[fsl].astype(bf)),
            "qcos": qcos, "qsin": qsin, "kcos": kcos, "ksin": ksin,
            "tri": tri, "ma": ma, "mm": mm, "r1c": r1c,
        })
    _CACHE["in_maps"] = in_maps
    res = run_bass_kernel_spmd(nc, in_maps, core_ids=list(range(NC)))
    _CACHE["res"] = res
    outT = np.concatenate([res.results[c]["out"] for c in range(NC)], axis=0)
    return np.ascontiguousarray(outT.T)[None]


if __name__ == "__main__":
    import reference
    inputs = reference.setup_inputs()
    out = kernel(**inputs)
    print(out.shape, out.dtype)


# revision 32
# speedup vs baseline: 1.6459x; 1.6459x over previous
"""Trainium2 Bass kernel for nn_LlamaMoDDecoderLayer (MoD decoder layer).

v2 strategy (8 NeuronCores, tensor-parallel, feature-major layouts):
  - Host precompute (free): router argmax masks in exact fp32; r1 (RMSNorm1
    row scales) folded into the RoPE cos/sin tables (q/k) and a per-token
    column vector (v); hsT shipped once in bf16; ln weights folded into Wq/
    Wk/Wv/w_gate/w_up.
  - QKV runs directly on raw bf16 hsT (the RMS column scale commutes out of
    the matmul); heads sharded 2/core; transposed-scores causal softmax with
    denominator accumulated on DVE (single ones-matmul per (h,chunk)).
  - Pipeline over 2 token-chunks of 1024: attn -> AG(ctx) -> Wo -> AG(hs2)
    -> norm2+MLP -> RS -> out, with per-chunk collectives overlapping PE.
  - MLP: w_gate/w_up column-sharded, w_down row-sharded, partial outputs
    ReduceScattered per chunk.
  - Matmuls bf16 (host-cast weights), fp32 PSUM accumulation; fp32 residual.
"""

import numpy as np
import ml_dtypes

import concourse.bass as bass
import concourse.bacc as bacc
import concourse.mybir as mybir
import concourse.tile as tile
from concourse.alu_op_type import AluOpType
from concourse.bass_utils import run_bass_kernel_spmd

F32 = mybir.dt.float32
BF16 = mybir.dt.bfloat16
FP8 = mybir.dt.float8e4
AF = mybir.ActivationFunctionType

S, D, H, Dh, F = 2048, 2048, 16, 128, 8192
NC = 8
HPC = H // NC            # heads per core (2)
DCC = D // NC            # output cols per core (256)
FPC = F // NC            # mlp hidden per core (1024)
NDT = D // 128           # 16 d-tiles
NFT = FPC // 128         # 8 local f-tiles
NSC = S // 512           # 4 s-chunks of 512
NPCA = 4                 # attention/AG1 pipeline chunks
SCA = S // NPCA          # tokens per attention chunk (512)
NPCM = 2                 # AG2/MLP/RS pipeline chunks
SCM = S // NPCM          # tokens per MLP chunk (1024)
NQCM = SCM // 512        # 512-token subchunks per MLP chunk
EPS = 1e-5
THETA = 10000.0

_CACHE = {}


def _build_program():
    nc = bacc.Bacc("TRN2", target_bir_lowering=False, debug=False,
                   num_devices=NC)
    rg = [list(range(NC))]

    d_bht = nc.dram_tensor("bht", [D, S], BF16, kind="ExternalInput")
    d_hres = nc.dram_tensor("hres", [DCC, S], F32, kind="ExternalInput")
    d_wq = nc.dram_tensor("wq", [D, DCC], BF16, kind="ExternalInput")
    d_wk = nc.dram_tensor("wk", [D, DCC], BF16, kind="ExternalInput")
    d_wv = nc.dram_tensor("wv", [D, DCC], BF16, kind="ExternalInput")
    d_wo = nc.dram_tensor("wo", [D, DCC], BF16, kind="ExternalInput")
    d_wg = nc.dram_tensor("wg", [D, FPC], BF16, kind="ExternalInput")
    d_wu = nc.dram_tensor("wu", [D, FPC], BF16, kind="ExternalInput")
    d_wd = nc.dram_tensor("wd", [FPC, D], BF16, kind="ExternalInput")
    d_qcos = nc.dram_tensor("qcos", [Dh, S], BF16, kind="ExternalInput")
    d_qsin = nc.dram_tensor("qsin", [Dh, S], BF16, kind="ExternalInput")
    d_kcos = nc.dram_tensor("kcos", [Dh, S], BF16, kind="ExternalInput")
    d_ksin = nc.dram_tensor("ksin", [Dh, S], BF16, kind="ExternalInput")
    d_tri = nc.dram_tensor("tri", [128, 4 * 512], BF16, kind="ExternalInput")
    d_ma = nc.dram_tensor("ma", [128, S], BF16, kind="ExternalInput")
    d_mm = nc.dram_tensor("mm", [128, S], BF16, kind="ExternalInput")
    d_r1c = nc.dram_tensor("r1c", [128, NDT], F32, kind="ExternalInput")
    d_out = nc.dram_tensor("out", [DCC, S], F32, kind="ExternalOutput")

    cc1i, cc1o, cc2i, cc2o, cc3i, cc3o = [], [], [], [], [], []
    for pc in range(NPCA):
        cc1i.append(nc.dram_tensor(f"cc1i{pc}", [DCC, SCA], FP8))
        cc1o.append(nc.dram_tensor(f"cc1o{pc}", [D, SCA], FP8,
                                   addr_space="Shared"))
    for pc in range(NPCM):
        cc2i.append(nc.dram_tensor(f"cc2i{pc}", [DCC, SCM], FP8))
        cc2o.append(nc.dram_tensor(f"cc2o{pc}", [D, SCM], FP8,
                                   addr_space="Shared"))
        cc3i.append(nc.dram_tensor(f"cc3i{pc}", [D, SCM], BF16))
        cc3o.append(nc.dram_tensor(f"cc3o{pc}", [DCC, SCM], BF16))

    bht_t = d_bht.ap().rearrange("(a p) s -> p a s", p=128)
    hres_t = d_hres.ap().rearrange("(a p) s -> p a s", p=128)
    wq_t = d_wq.ap().rearrange("(a p) m -> p a m", p=128)
    wk_t = d_wk.ap().rearrange("(a p) m -> p a m", p=128)
    wv_t = d_wv.ap().rearrange("(a p) m -> p a m", p=128)
    wo_t = d_wo.ap().rearrange("(a p) m -> p a m", p=128)
    wg_t = d_wg.ap().rearrange("(a p) m -> p a m", p=128)
    wu_t = d_wu.ap().rearrange("(a p) m -> p a m", p=128)
    wd_t = d_wd.ap().rearrange("(a p) m -> p a m", p=128)
    cc1i_t = [t.ap().rearrange("(a p) s -> p a s", p=128) for t in cc1i]
    cc1o_t = [t.ap().rearrange("(a p) s -> p a s", p=128) for t in cc1o]
    cc2i_t = [t.ap().rearrange("(a p) s -> p a s", p=128) for t in cc2i]
    cc2o_t = [t.ap().rearrange("(a p) s -> p a s", p=128) for t in cc2o]
    cc3i_t = [t.ap().rearrange("(a p) s -> p a s", p=128) for t in cc3i]
    cc3o_t = [t.ap().rearrange("(a p) s -> p a s", p=128) for t in cc3o]
    out_t = d_out.ap().rearrange("(a p) s -> p a s", p=128)

    with tile.TileContext(nc) as tc:
        with (
            tc.tile_pool(name="const", bufs=1) as cst,
            tc.tile_pool(name="persist", bufs=1) as pst,
            tc.tile_pool(name="psum", bufs=2, space="PSUM") as psp,
        ):
            ones_b = cst.tile([128, 1], BF16)
            nc.gpsimd.memset(ones_b[:], 1.0)
            ones_r = cst.tile([1, 128], F32)
            nc.gpsimd.memset(ones_r[:], 1.0)
            eps1 = cst.tile([1, 1], F32)
            nc.gpsimd.memset(eps1[:], EPS)
            r1c = cst.tile([128, NDT], F32, name="r1c")
            nc.sync.dma_start(r1c[:], d_r1c.ap())
            ma_b = pst.tile([128, S], BF16, name="ma_b")
            mm_b = pst.tile([128, S], BF16, name="mm_b")
            nc.sync.dma_start(ma_b[:], d_ma.ap())
            nc.sync.dma_start(mm_b[:], d_mm.ap())
            # hs2f doubles as the residual: loaded with hres, Wo adds into it
            hs2f = pst.tile([128, HPC, S], F32, name="hs2f")
            nc.sync.dma_start(hs2f[:], hres_t)
            wo = pst.tile([128, NDT, DCC], BF16, name="wo")
            nc.scalar.dma_start(wo[:], wo_t)

            with tc.tile_pool(name="attn", bufs=1) as atp:
                qr = atp.tile([128, HPC, S], BF16, name="qr")
                kr = atp.tile([128, HPC, S], BF16, name="kr")
                v_sb = atp.tile([128, NDT, DCC], BF16, name="v_sb")
                tri = atp.tile([128, 4, 512], BF16, name="tri")
                nc.sync.dma_start(
                    tri[:], d_tri.ap().rearrange("p (a m) -> p a m", m=512))
                q_sb = atp.tile([128, HPC, S], BF16, name="q_sb")
                k_sb = atp.tile([128, HPC, S], BF16, name="k_sb")
                qcos = atp.tile([128, S], BF16, name="qcos")
                qsin = atp.tile([128, S], BF16, name="qsin")
                kcos = atp.tile([128, S], BF16, name="kcos")
                ksin = atp.tile([128, S], BF16, name="ksin")
                nc.sync.dma_start(qcos[:], d_qcos.ap())
                nc.sync.dma_start(qsin[:], d_qsin.ap())
                nc.sync.dma_start(kcos[:], d_kcos.ap())
                nc.sync.dma_start(ksin[:], d_ksin.ap())

                # ---- per half: QKV on raw bht -> rope -> attention -> AG1.
                #      r1 folded into rope tables (q,k) and r1c (v) ----
                with tc.tile_pool(name="qk", bufs=1) as qkp:
                    wq = qkp.tile([128, NDT, DCC], BF16, name="wq")
                    wk = qkp.tile([128, NDT, DCC], BF16, name="wk")
                    wv = qkp.tile([128, NDT, DCC], BF16, name="wv")
                    nc.scalar.dma_start(wq[:], wq_t)
                    nc.scalar.dma_start(wk[:], wk_t)
                    nc.scalar.dma_start(wv[:], wv_t)
                    qs_sb = qkp.tile([128, HPC, S], BF16, name="qs_sb")
                    ks_sb = qkp.tile([128, HPC, S], BF16, name="ks_sb")
                    for pc in range(NPCA):
                        cols = slice(pc * SCA, (pc + 1) * SCA)
                        bh = qkp.tile([128, NDT, SCA], BF16, tag="bht",
                                      bufs=1)
                        for a in range(NDT):
                            nc.sync.dma_start(
                                bh[:, a, :], bht_t[:, a, bass.ts(pc, SCA)])
                        for w_sb, t_sb in ((wq, q_sb), (wk, k_sb)):
                            for mc in range(HPC):
                                ps = psp.tile([128, 512], F32, tag="mmps")
                                for a in range(NDT):
                                    nc.tensor.matmul(
                                        ps[:],
                                        w_sb[:, a, bass.ts(mc, 128)],
                                        bh[:, a, :],
                                        start=(a == 0),
                                        stop=(a == NDT - 1))
                                nc.scalar.copy(
                                    t_sb[:, mc, bass.ts(pc, 512)], ps[:])
                        for mi in range(NDT // NPCA):
                            mc = pc * (NDT // NPCA) + mi
                            ps = psp.tile([128, DCC], F32, tag="vps", bufs=1)
                            for a in range(NDT):
                                nc.tensor.matmul(
                                    ps[:], bh[:, a, bass.ts(mi, 128)],
                                    wv[:, a, :],
                                    start=(a == 0), stop=(a == NDT - 1))
                            nc.vector.tensor_scalar(
                                v_sb[:, mc, :], ps[:], r1c[:, mc:mc + 1],
                                None, op0=AluOpType.mult)
                        # rope for this half
                        for src, dst in ((q_sb, qs_sb), (k_sb, ks_sb)):
                            for mc in range(HPC):
                                nc.sync.dma_start(dst[0:64, mc, cols],
                                                  src[64:128, mc, cols])
                                nc.sync.dma_start(dst[64:128, mc, cols],
                                                  src[0:64, mc, cols])
                        for mc in range(HPC):
                            tq = qkp.tile([128, SCA], BF16, tag="ropetmp",
                                          bufs=2)
                            nc.vector.tensor_tensor(tq[:], qs_sb[:, mc, cols],
                                                    qsin[:, cols],
                                                    op=AluOpType.mult)
                            nc.vector.tensor_tensor(qr[:, mc, cols],
                                                    q_sb[:, mc, cols],
                                                    qcos[:, cols],
                                                    op=AluOpType.mult)
                            nc.vector.tensor_tensor(qr[:, mc, cols],
                                                    qr[:, mc, cols], tq[:],
                                                    op=AluOpType.add)
                            tk = qkp.tile([128, SCA], BF16, tag="ropetmp",
                                          bufs=2)
                            nc.vector.tensor_tensor(tk[:], ks_sb[:, mc, cols],
                                                    ksin[:, cols],
                                                    op=AluOpType.mult)
                            nc.vector.tensor_tensor(kr[:, mc, cols],
                                                    k_sb[:, mc, cols],
                                                    kcos[:, cols],
                                                    op=AluOpType.mult)
                            nc.vector.tensor_tensor(kr[:, mc, cols],
                                                    kr[:, mc, cols], tk[:],
                                                    op=AluOpType.add)
                        # attention for this half's two q-subchunks
                        for qs_i in range(NQC):
                        qc = pc * NQC + qs_i
                        for h in range(HPC):
                            nkt = 4 * (qc + 1)
                            cps = psp.tile([128, 512], F32, tag="ctxps",
                                           bufs=1)
                            dsum = atp.tile([128, 512], BF16, tag="dsum",
                                            bufs=2)
                            for kt in range(nkt):
                                sps = psp.tile([128, 512], F32, tag="stps")
                                nc.tensor.matmul(sps[:],
                                                 kr[:, h, bass.ts(kt, 128)],
                                                 qr[:, h, bass.ts(qc, 512)])
                                est = atp.tile([128, 512], BF16, tag="est",
                                               bufs=3)
                                nc.scalar.activation(est[:], sps[:], AF.Exp)
                                if kt // 4 == qc:
                                    nc.vector.tensor_tensor(
                                        est[:], est[:], tri[:, kt % 4, :],
                                        op=AluOpType.mult)
                                nc.tensor.matmul(cps[:],
                                                 v_sb[:, kt, bass.ts(h, 128)],
                                                 est[:], start=(kt == 0),
                                                 stop=(kt == nkt - 1))
                                if kt == 0:
                                    nc.vector.tensor_copy(dsum[:], est[:])
                                else:
                                    nc.vector.tensor_tensor(
                                        dsum[:], dsum[:], est[:],
                                        op=AluOpType.add)
                            dps = psp.tile([1, 512], F32, tag="rowps",
                                           bufs=1)
                            nc.tensor.matmul(dps[:], ones_b[:], dsum[:])
                            rrow = atp.tile([1, 512], F32, tag="rrow", bufs=2)
                            nc.vector.reciprocal(rrow[:], dps[:])
                            bps = psp.tile([128, 512], F32, tag="bcps",
                                           bufs=1)
                            nc.tensor.matmul(bps[:], ones_r[:], rrow[:])
                            rb = atp.tile([128, 512], F32, tag="rb", bufs=2)
                            nc.scalar.copy(rb[:], bps[:])
                            ctxc = atp.tile([128, 512], BF16, tag="ctxc",
                                            bufs=2)
                            nc.vector.tensor_tensor(ctxc[:], cps[:], rb[:],
                                                    op=AluOpType.mult)
                            nc.sync.dma_start(
                                cc1i_t[pc][:, h, bass.ts(qs_i, 512)],
                                ctxc[:])
                    nc.gpsimd.collective_compute(
                        "AllGather", AluOpType.bypass, replica_groups=rg,
                        ins=[cc1i[pc].ap()], outs=[cc1o[pc].ap()])

            # attention SBUF freed: stream MLP weights into that space
            with tc.tile_pool(name="mlw", bufs=1) as mlw:
                wg = mlw.tile([128, NDT, FPC], BF16, name="wg")
                wu = mlw.tile([128, NDT, FPC], BF16, name="wu")

                # ---- Wo + hs2 for the remaining chunks ----
                wo_chunk(2)
                wo_chunk(3)

                # ---- norm2 + MLP + RS + out per chunk ----
                with tc.tile_pool(name="mlp", bufs=1) as mlp:
                    for pc in range(NPCM):
                        hs2r = mlp.tile([128, NDT, SCM], FP8, tag="hs2r",
                                        bufs=1)
                        nc.sync.dma_start(hs2r[:], cc2o_t[pc])
                        hs2g = mlp.tile([128, NDT, SCM], BF16, tag="hs2g",
                                        bufs=1)
                        for a in range(NDT):
                            bh2 = mlp.tile([128, SCM], BF16, tag="bh2",
                                           bufs=3)
                            nc.sync.dma_start(bh2[:],
                                              bht_t[:, a, bass.ts(pc, SCM)])
                            nc.vector.tensor_tensor(hs2g[:, a, :],
                                                    hs2r[:, a, :], bh2[:],
                                                    op=AluOpType.add)
                        r2b = mlp.tile([128, SCM], F32, tag="r2b", bufs=1)
                        for sc_i in range(NQCM):
                            rps = psp.tile([1, 512], F32, tag="rowps",
                                           bufs=1)
                            for a in range(NDT):
                                sq = mlp.tile([128, 512], BF16, tag="sq",
                                              bufs=2)
                                nc.scalar.activation(
                                    sq[:], hs2g[:, a, bass.ts(sc_i, 512)],
                                    AF.Square)
                                nc.tensor.matmul(rps[:], ones_b[:], sq[:],
                                                 start=(a == 0),
                                                 stop=(a == NDT - 1))
                            r2row = mlp.tile([1, 512], F32, tag="r2row",
                                             bufs=2)
                            nc.scalar.activation(r2row[:], rps[:], AF.Sqrt,
                                                 bias=eps1[:], scale=1.0 / D)
                            nc.vector.reciprocal(r2row[:], r2row[:])
                            bps = psp.tile([128, 512], F32, tag="bcps",
                                           bufs=1)
                            nc.tensor.matmul(bps[:], ones_r[:], r2row[:])
                            nc.scalar.copy(r2b[:, bass.ts(sc_i, 512)], bps[:])
                        hT = mlp.tile([128, NFT, SCM], BF16, tag="hT", bufs=1)
                        for fc in range(NFT):
                            sg = mlp.tile([128, SCM], BF16, tag="sg", bufs=1)
                            for sc_i in range(NQCM):
                                ps = psp.tile([128, 512], F32, tag="mmps")
                                for a in range(NDT):
                                    nc.tensor.matmul(
                                        ps[:], wg[:, a, bass.ts(fc, 128)],
                                        hs2g[:, a, bass.ts(sc_i, 512)],
                                        start=(a == 0), stop=(a == NDT - 1))
                                gsc = mlp.tile([128, 512], BF16, tag="gsc",
                                               bufs=1)
                                nc.vector.tensor_tensor(
                                    gsc[:], ps[:],
                                    r2b[:, bass.ts(sc_i, 512)],
                                    op=AluOpType.mult)
                                nc.scalar.activation(
                                    sg[:, bass.ts(sc_i, 512)], gsc[:],
                                    AF.Silu)
                            ssg = mlp.tile([128, SCM], BF16, tag="ssg",
                                           bufs=2)
                            nc.vector.tensor_tensor(ssg[:], sg[:], r2b[:],
                                                    op=AluOpType.mult)
                            for sc_i in range(NQCM):
                                ps = psp.tile([128, 512], F32, tag="mmps")
                                for a in range(NDT):
                                    nc.tensor.matmul(
                                        ps[:], wu[:, a, bass.ts(fc, 128)],
                                        hs2g[:, a, bass.ts(sc_i, 512)],
                                        start=(a == 0), stop=(a == NDT - 1))
                                nc.vector.tensor_tensor(
                                    hT[:, fc, bass.ts(sc_i, 512)], ps[:],
                                    ssg[:, bass.ts(sc_i, 512)],
                                    op=AluOpType.mult)
                        for mc in range(NDT):
                            wdc = mlp.tile([128, NFT, 128], BF16, tag="wdc",
                                           bufs=3)
                            nc.scalar.dma_start(wdc[:],
                                                wd_t[:, :, bass.ts(mc, 128)])
                            for sc_i in range(NQCM):
                                ps = psp.tile([128, 512], F32, tag="mmps")
                                for a in range(NFT):
                                    nc.tensor.matmul(
                                        ps[:], wdc[:, a, :],
                                        hT[:, a, bass.ts(sc_i, 512)],
                                        start=(a == 0), stop=(a == NFT - 1))
                                stg = mlp.tile([128, 512], BF16, tag="stg",
                                               bufs=2)
                                nc.scalar.copy(stg[:], ps[:])
                                nc.sync.dma_start(
                                    cc3i_t[pc][:, mc, bass.ts(sc_i, 512)],
                                    stg[:])
                        nc.gpsimd.collective_compute(
                            "ReduceScatter", AluOpType.add, replica_groups=rg,
                            ins=[cc3i[pc].ap()], outs=[cc3o[pc].ap()])

                        rs = mlp.tile([128, HPC, SCM], BF16, tag="rs", bufs=2)
                        nc.sync.dma_start(rs[:], cc3o_t[pc])
                        for mc in range(HPC):
                            col = pc * SCM
                            t2 = mlp.tile([128, SCM], F32, tag="fint", bufs=2)
                            nc.vector.tensor_tensor(t2[:], rs[:, mc, :],
                                                    mm_b[:, col:col + SCM],
                                                    op=AluOpType.mult)
                            nc.vector.tensor_tensor(
                                t2[:], t2[:], hs2f[:, mc, col:col + SCM],
                                op=AluOpType.add)
                            nc.sync.dma_start(
                                out_t[:, mc, col:col + SCM], t2[:])

    nc.compile()
    return nc


def _rope_tables():
    pos = np.arange(S, dtype=np.float32)
    inv = 1.0 / (THETA ** (np.arange(0, Dh, 2, dtype=np.float32) / Dh))
    ang = pos[:, None] * inv[None, :]
    emb = np.concatenate([ang, ang], axis=-1)          # [S, Dh]
    cosT = np.cos(emb).T.astype(np.float32).copy()     # [Dh, S]
    ssinT = np.sin(emb).T.astype(np.float32).copy()
    ssinT[:64] = -ssinT[:64]
    return cosT, ssinT


def _tri_masks():
    # [128, 4, 512] for the diagonal 512-q-chunk, k-tile offset i in chunk:
    # col j: 0 if j < 128i; causal tri inside diag block; 1 past it.
    m = np.zeros((128, 4, 512), np.float32)
    for i in range(4):
        j = np.arange(512)[None, :]
        p = np.arange(128)[:, None]
        m[:, i, :] = ((j - 128 * i) >= p).astype(np.float32)
        m[:, i, : 128 * i] = 0.0
        m[:, i, 128 * (i + 1):] = 1.0
    return m.reshape(128, 4 * 512)


def kernel(**inputs):
    bf = ml_dtypes.bfloat16
    hs = np.ascontiguousarray(np.asarray(inputs["hidden_states"],
                                         np.float32)[0])
    ln1 = np.asarray(inputs["ln1_w"], np.float32)
    ln2 = np.asarray(inputs["ln2_w"], np.float32)
    Wq = np.asarray(inputs["Wq"], np.float32) * ln1[:, None]
    Wk = np.asarray(inputs["Wk"], np.float32) * ln1[:, None]
    Wv = np.asarray(inputs["Wv"], np.float32) * ln1[:, None]
    Wo = np.asarray(inputs["Wo"], np.float32)
    wg = np.asarray(inputs["w_gate"], np.float32) * ln2[:, None]
    wu = np.asarray(inputs["w_up"], np.float32) * ln2[:, None]
    wd = np.asarray(inputs["w_down"], np.float32)
    raw = np.asarray(inputs["router_attn_w"], np.float32)
    rab = np.asarray(inputs["router_attn_b"], np.float32)
    rmw = np.asarray(inputs["router_mlp_w"], np.float32)
    rmb = np.asarray(inputs["router_mlp_b"], np.float32)

    hsT = np.ascontiguousarray(hs.T)                   # [D, S]

    # routers on host, exact fp32 semantics (keep = argmax == 0)
    al = hs @ raw + rab
    ml_ = hs @ rmw + rmb
    keep_a = (al[:, 1] <= al[:, 0]).astype(np.float32)      # [S]
    keep_m = (ml_[:, 1] <= ml_[:, 0]).astype(np.float32)
    ma = np.ascontiguousarray(
        np.broadcast_to(keep_a[None, :], (128, S)).astype(bf))
    mm = np.ascontiguousarray(
        np.broadcast_to(keep_m[None, :], (128, S)).astype(bf))

    # RMSNorm1 row scales, folded into rope tables (q,k) and r1c (v)
    r1 = (1.0 / np.sqrt((hsT * hsT).mean(0) + EPS)).astype(np.float32)  # [S]
    cosT, ssinT = _rope_tables()
    sc = np.float32(1.0 / np.sqrt(Dh))
    qcos = np.ascontiguousarray((cosT * r1[None, :]).astype(bf))
    qsin = np.ascontiguousarray((ssinT * r1[None, :]).astype(bf))
    kcos = np.ascontiguousarray((cosT * (r1 * sc)[None, :]).astype(bf))
    ksin = np.ascontiguousarray((ssinT * (r1 * sc)[None, :]).astype(bf))
    r1c = np.ascontiguousarray(r1.reshape(NDT, 128).T)  # [128, 16]

    tri = np.ascontiguousarray(_tri_masks().astype(bf))
    bht = np.ascontiguousarray(hsT.astype(bf))

    if "nc" not in _CACHE:
        _CACHE["nc"] = _build_program()
    nc = _CACHE["nc"]

    in_maps = []
    for c in range(NC):
        dsl = slice(c * DCC, (c + 1) * DCC)
        fsl = slice(c * FPC, (c + 1) * FPC)
        in_maps.append({
            "bht": bht,
            "hres": np.ascontiguousarray(hsT[dsl]),
            "wq": np.ascontiguousarray(Wq[:, dsl].astype(bf)),
            "wk": np.ascontiguousarray(Wk[:, dsl].astype(bf)),
            "wv": np.ascontiguousarray(Wv[:, dsl].astype(bf)),
            "wo": np.ascontiguousarray(Wo[:, dsl].astype(bf)),
            "wg": np.ascontiguousarray(wg[:, fsl].astype(bf)),
            "wu": np.ascontiguousarray(wu[:, fsl].astype(bf)),
            "wd": np.ascontiguousarray(wd[fsl].astype(bf)),
            "qcos": qcos, "qsin": qsin, "kcos": kcos, "ksin": ksin,
            "tri": tri, "ma": ma, "mm": mm, "r1c": r1c,
        })
    _CACHE["in_maps"] = in_maps
    res = run_bass_kernel_spmd(nc, in_maps, core_ids=list(range(NC)))
    _CACHE["res"] = res
    outT = np.concatenate([res.results[c]["out"] for c in range(NC)], axis=0)
    return np.ascontiguousarray(outT.T)[None]


if __name__ == "__main__":
    import reference
    inputs = reference.setup_inputs()
    out = kernel(**inputs)
    print(out.shape, out.dtype)
